# revision 2
# baseline (speedup 1.0000x reference)
"""Trainium2 Bass kernel v3 for nn_Gemma3MoEAttention (B=4,T=2048,D=2048,NH=8,NKV=4,HD=256).

Sharding: core c -> (batch c//2, kv-head-half c%2). Each core computes
K/V for its 2 kv heads, Q for its 4 q heads over the FULL sequence,
flash-free attention in transposed layout (S^T[w,q], P@V -> encT
directly), and a row-sharded out-projection partial [T, D]. The host
sums the two partials per batch during unshard (the "all-reduce" of the
row-sharded out_proj). Zero duplicated FLOPs across cores; all matmuls
bf16; everything SBUF-resident between phases.
"""
import numpy as np
import ml_dtypes

import concourse.bass as bass
import concourse.mybir as mybir
import concourse.tile as tile
from concourse import bacc

F32 = mybir.dt.float32
F32R = mybir.dt.float32r
BF16 = mybir.dt.bfloat16

B, T, D = 4, 2048, 2048
NH, NKV, HD = 8, 4, 256
P = 128
NBLK = T // P          # 16 t blocks
NHL = 2                # kv heads per core
NQH = 4                # q heads per core
NG = 8                 # q groups (pairs of blocks) per core
DC = D // P            # 16 contraction chunks
ROPE_BASE = 10000.0
SOFT_CAP = 50.0
EXP_SCALE = SOFT_CAP / float(np.sqrt(HD))   # 3.125
MASK_NEG = -60.0
ALU = mybir.AluOpType
ACTF = mybir.ActivationFunctionType


def build_nc(group_L, group_P):
    """group_L[g]: kv extent (128-blocks, even) for q-block pair g=(2g,2g+1).
    group_P[g]: leading all-True kv blocks (mask skipped)."""
    assert len(group_L) == NG and len(group_P) == NG
    mcols = sum((L - P0) * 256 for L, P0 in zip(group_L, group_P))
    nc = bacc.Bacc("TRN2", target_bir_lowering=False, debug=False, num_devices=8)

    xT = nc.declare_dram_parameter("xT", [D, T], BF16, isOutput=False)
    wq = nc.declare_dram_parameter("wq", [D, NQH * HD], BF16, isOutput=False)
    wk = nc.declare_dram_parameter("wk", [D, NHL * HD], BF16, isOutput=False)
    wv = nc.declare_dram_parameter("wv", [D, NHL * HD], BF16, isOutput=False)
    wo = nc.declare_dram_parameter("wo", [NQH * HD, D], BF16, isOutput=False)
    cos_k = nc.declare_dram_parameter("cos_k", [P, T], BF16, isOutput=False)
    sin_k = nc.declare_dram_parameter("sin_k", [P, T], BF16, isOutput=False)
    maskT = nc.declare_dram_parameter("maskT", [P, max(mcols, 256)], BF16,
                                      isOutput=False)
    c1bf = nc.declare_dram_parameter("c1bf", [P, 8], BF16, isOutput=False)
    c1fr = nc.declare_dram_parameter("c1fr", [1, P], F32R, isOutput=False)
    out = nc.declare_dram_parameter("out", [T, D], F32, isOutput=True)

    a = dict(
        xT_r=xT.rearrange("(o p) t -> p o t", p=P),
        wq_r=wq.rearrange("(o p) c -> p o c", p=P),
        wk_r=wk.rearrange("(o p) c -> p o c", p=P),
        wv_r=wv.rearrange("(o p) c -> p o c", p=P),
        wo_r=wo.rearrange("(o p) c -> p o c", p=P),
        out_r=out.rearrange("(s p) d -> p s d", p=P),
        cos_k=cos_k, sin_k=sin_k, maskT=maskT, c1bf=c1bf, c1fr=c1fr,
    )

    with tile.TileContext(nc) as tc:
        _emit_body(nc, tc, a, group_L, group_P)
    nc.finalize()
    return nc


def _rope6(nc, rp, p0_bf, p1_bf, cos_ap, sin_ap, dst0, dst1):
    """dst0 = p0*cos - p1*sin ; dst1 = p1*cos + p0*sin. All bf16 SBUF."""
    tA = rp.tile([P, 512], BF16, tag="rA")
    nc.vector.tensor_tensor(tA[:], p0_bf, cos_ap, ALU.mult)
    tB = rp.tile([P, 512], BF16, tag="rB")
    nc.vector.tensor_tensor(tB[:], p1_bf, sin_ap, ALU.mult)
    nc.vector.tensor_tensor(dst0, tA[:], tB[:], ALU.subtract)
    tC = rp.tile([P, 512], BF16, tag="rC")
    nc.vector.tensor_tensor(tC[:], p1_bf, cos_ap, ALU.mult)
    tD = rp.tile([P, 512], BF16, tag="rD")
    nc.vector.tensor_tensor(tD[:], p0_bf, sin_ap, ALU.mult)
    nc.vector.tensor_tensor(dst1, tC[:], tD[:], ALU.add)


def _emit_body(nc, tc, a, group_L, group_P):
    with (
        tc.tile_pool(name="kT_pool", bufs=1) as kTp,
        tc.tile_pool(name="v_pool", bufs=1) as vp_,
        tc.tile_pool(name="qT_pool", bufs=1) as qTp,
        tc.tile_pool(name="trig", bufs=1) as ktp,
    ):
        kT = kTp.tile([P, 2 * NHL, T], BF16)          # [hd128, chunk, t]
        v_sb = vp_.tile([P, NBLK, NHL * HD], BF16)    # [t128, tblk, feat]
        qT = qTp.tile([P, 2 * NQH, T], BF16)          # [hd128, chunk, t]

        # ---------------- Phase 1: K/V/Q projections + rope -----------------
        with (
            tc.tile_pool(name="xt_pool", bufs=1) as xtp,
            tc.tile_pool(name="drain", bufs=2) as drp,
            tc.tile_pool(name="rope", bufs=1) as rp,
            tc.tile_pool(name="wkq_pool", bufs=2) as wsp,
            tc.tile_pool(name="wv_pool", bufs=2) as wvp,
        ):
            # all phase-1 DMAs up front: wk, xT stream, trig, wv, wq chunks
            wk0 = wsp.tile([P, DC, 2 * P], BF16, tag="wq", name="wk0")
            nc.sync.dma_start(wk0[:], a["wk_r"][:, :, 0:2 * P])
            xT_d = []
            for d in range(DC):
                t_ = xtp.tile([P, T], BF16, tag=f"xT{d}", name=f"xT{d}")
                nc.sync.dma_start(t_[:], a["xT_r"][:, d, :])
                xT_d.append(t_)
            cosk = ktp.tile([P, T], BF16)
            nc.sync.dma_start(cosk[:], a["cos_k"][:])
            sink = ktp.tile([P, T], BF16)
            nc.sync.dma_start(sink[:], a["sin_k"][:])


            # K proj: kT[feat128, t]; rope chunk pairs (2k, 2k+1)
            with tc.tile_pool(name="k_psum", bufs=3, space="PSUM") as kps:
                for kv in range(NHL):
                    if kv == 0:
                        wk_sb = wk0
                    else:
                        wk_sb = wsp.tile([P, DC, 2 * P], BF16, tag="wq",
                                         name="wk1")
                        nc.sync.dma_start(
                            wk_sb[:], a["wk_r"][:, :, HD * kv:HD * (kv + 1)])
                    for tg in range(4):
                        ts_ = slice(512 * tg, 512 * (tg + 1))
                        ps = [kps.tile([P, 512], F32, tag=f"kps{i}",
                                       name=f"kps{i}", bufs=3)
                              for i in range(2)]
                        for d in range(DC):
                            for i in range(2):
                                nc.tensor.matmul(
                                    ps[i][:],
                                    wk_sb[:, d, P * i:P * (i + 1)],
                                    xT_d[d][:, ts_],
                                    start=(d == 0), stop=(d == DC - 1))
                        b0 = drp.tile([P, 512], BF16, tag="db0")
                        nc.scalar.copy(b0[:], ps[0][:])
                        b1 = drp.tile([P, 512], BF16, tag="db1")
                        nc.scalar.copy(b1[:], ps[1][:])
                        _rope6(nc, rp, b0[:], b1[:], cosk[:, ts_], sink[:, ts_],
                               kT[:, 2 * kv, ts_], kT[:, 2 * kv + 1, ts_])

            # V proj: v[t128, feat], two 256-wide feature halves
            with tc.tile_pool(name="v_psum", bufs=3, space="PSUM") as vps:
                for vh in range(2):
                    wv_sb = wvp.tile([P, DC, 2 * P], BF16, tag="wv")
                    nc.sync.dma_start(
                        wv_sb[:], a["wv_r"][:, :, 2 * P * vh:2 * P * (vh + 1)])
                    for tb in range(NBLK):
                        ps_v = vps.tile([P, 2 * P], F32, tag="vps", bufs=3)
                        for d in range(DC):
                            nc.tensor.matmul(
                                ps_v[:], xT_d[d][:, P * tb:P * (tb + 1)],
                                wv_sb[:, d, :],
                                start=(d == 0), stop=(d == DC - 1))
                        nc.scalar.copy(
                            v_sb[:, tb, 2 * P * vh:2 * P * (vh + 1)], ps_v[:])

            # Q proj: qT[feat128, t] over the full sequence; rope
            with tc.tile_pool(name="q_psum", bufs=3, space="PSUM") as qps:
                for h in range(NQH):
                    wq_sb = wsp.tile([P, DC, 2 * P], BF16, tag="wq")
                    nc.sync.dma_start(
                        wq_sb[:], a["wq_r"][:, :, 2 * P * h:2 * P * (h + 1)])
                    for tg in range(4):
                        ts_ = slice(512 * tg, 512 * (tg + 1))
                        ps = [qps.tile([P, 512], F32, tag=f"qps{i}",
                                       name=f"qps{i}", bufs=3)
                              for i in range(2)]
                        for d in range(DC):
                            for i in range(2):
                                nc.tensor.matmul(
                                    ps[i][:], wq_sb[:, d, P * i:P * (i + 1)],
                                    xT_d[d][:, ts_],
                                    start=(d == 0), stop=(d == DC - 1))
                        b0 = drp.tile([P, 512], BF16, tag="db0")
                        nc.scalar.copy(b0[:], ps[0][:])
                        b1 = drp.tile([P, 512], BF16, tag="db1")
                        nc.scalar.copy(b1[:], ps[1][:])
                        _rope6(nc, rp, b0[:], b1[:], cosk[:, ts_], sink[:, ts_],
                               qT[:, 2 * h, ts_], qT[:, 2 * h + 1, ts_])

        # ---------------- Phase 2: attention (S transposed) -----------------
        with (
            tc.tile_pool(name="encT_pool", bufs=1) as ep_,
            tc.tile_pool(name="attn_const", bufs=1) as acp,
            tc.tile_pool(name="mask_pool", bufs=1) as mkp,
            tc.tile_pool(name="wo_pool", bufs=1) as wop,
        ):
            encT = ep_.tile([P, 2 * NQH, T], BF16)    # [hd128, chunk, qcol]
            ones_bf = acp.tile([P, 8], BF16)
            nc.sync.dma_start(ones_bf[:], a["c1bf"][:])
            ones_fr = acp.tile([1, P], F32R)
            nc.sync.dma_start(ones_fr[:], a["c1fr"][:])
            mk_sb = []
            off = 0
            for g in range(NG):
                w = (group_L[g] - group_P[g]) * 256
                if w > 0:
                    m = mkp.tile([P, w], BF16, tag=f"mk{g}", name=f"mk{g}")
                    nc.sync.dma_start(m[:], a["maskT"][:, off:off + w])
                    mk_sb.append(m)
                    off += w
                else:
                    mk_sb.append(None)
            # prefetch wo during attention (SBUF has room in this sharding)
            wo_sb = wop.tile([P, 2 * NQH, D], BF16)
            nc.sync.dma_start(wo_sb[:], a["wo_r"][:])

            with (
                tc.tile_pool(name="st_psum", bufs=2, space="PSUM") as stp,
                tc.tile_pool(name="pv_psum", bufs=2, space="PSUM") as pvp,
                tc.tile_pool(name="dr_psum", bufs=2, space="PSUM") as drps,
                tc.tile_pool(name="scr_pool", bufs=3) as scp,
                tc.tile_pool(name="pt_pool", bufs=1) as ptp,
                tc.tile_pool(name="small", bufs=4) as smp,
            ):
                def emit_st(h, g):
                    kv = h // 2
                    L, P0 = group_L[g], group_P[g]
                    qs = slice(256 * g, 256 * (g + 1))
                    pt = ptp.tile([P, NBLK * 256], BF16, tag="PT", bufs=2)
                    lq = 0
                    while lq < L:
                        nb = min(4, L - lq)
                        ps_st = stp.tile([P, 4, 256], F32, tag="st", bufs=2)
                        for j in range(nb):
                            for c in range(2):
                                nc.tensor.matmul(
                                    ps_st[:, j, :],
                                    kT[:, 2 * kv + c, P * (lq + j):P * (lq + j + 1)],
                                    qT[:, 2 * h + c, qs],
                                    start=(c == 0), stop=(c == 1))
                        scr = scp.tile([P, 4, 256], F32, tag="scr")
                        nc.scalar.activation(
                            scr[:, :nb, :], ps_st[:, :nb, :], ACTF.Tanh,
                            scale=1.0 / SOFT_CAP)
                        for j in range(nb):
                            l = lq + j
                            if l >= P0:
                                mo = (l - P0) * 256
                                nc.vector.tensor_tensor(
                                    scr[:, j, :], scr[:, j, :],
                                    mk_sb[g][:, mo:mo + 256], ALU.add)
                        nc.scalar.activation(
                            pt[:, 256 * lq:256 * (lq + nb)], scr[:, :nb, :],
                            ACTF.Exp, scale=EXP_SCALE)
                        lq += nb
                    return pt

                def emit_pv(h, g, pt):
                    kv = h // 2
                    L = group_L[g]
                    qs = slice(256 * g, 256 * (g + 1))
                    ps_pv = pvp.tile([P, 2, 256], F32, tag="pv", bufs=2)
                    ps_dr = drps.tile([P, 512], F32, tag="dr", bufs=2)
                    for c in range(2):
                        for l in range(L):
                            nc.tensor.matmul(
                                ps_pv[:, c, :],
                                v_sb[:, l, HD * kv + P * c:HD * kv + P * (c + 1)],
                                pt[:, 256 * l:256 * (l + 1)],
                                start=(l == 0), stop=(l == L - 1))
                    for l in range(L):
                        nc.tensor.matmul(
                            ps_dr[:1, 0:256], ones_bf[:, 0:1],
                            pt[:, 256 * l:256 * (l + 1)],
                            start=(l == 0), stop=(l == L - 1))
                    rcp = smp.tile([1, 256], F32R, tag="rcp")
                    with nc.allow_low_precision(reason="f32r==f32 bits for "
                                                "broadcast matmul"):
                        nc.vector.reciprocal(rcp[:], ps_dr[:1, 0:256])
                    nc.tensor.matmul(ps_dr[:, 256:512], ones_fr[:1, :], rcp[:],
                                     start=True, stop=True)
                    rr = smp.tile([P, 256], F32, tag="rr")
                    nc.scalar.copy(rr[:], ps_dr[:, 256:512])
                    for c in range(2):
                        nc.vector.tensor_tensor(
                            encT[:, 2 * h + c, qs], ps_pv[:, c, :],
                            rr[:], ALU.mult)

                prev = None
                for h in range(NQH):
                    for g in range(NG):
                        pt = emit_st(h, g)
                        if prev is not None:
                            emit_pv(*prev)
                        prev = (h, g, pt)
                emit_pv(*prev)

            # ---------------- Phase 3: out projection (row-sharded) ---------
            with (
                tc.tile_pool(name="o_psum", bufs=3, space="PSUM") as ops,
                tc.tile_pool(name="o_drain", bufs=3) as odp,
            ):
                for s in range(NBLK):
                    for dq in range(4):
                        ps_o = ops.tile([P, 512], F32, tag="ops", bufs=3)
                        for fc in range(2 * NQH):
                            nc.tensor.matmul(
                                ps_o[:], encT[:, fc, P * s:P * (s + 1)],
                                wo_sb[:, fc, 512 * dq:512 * (dq + 1)],
                                start=(fc == 0), stop=(fc == 2 * NQH - 1))
                        o_sb = odp.tile([P, 512], F32, tag="osb")
                        nc.scalar.copy(o_sb[:], ps_o[:])
                        nc.sync.dma_start(
                            a["out_r"][:, s, 512 * dq:512 * (dq + 1)], o_sb[:])


# ======================= host-side glue ====================================

def plan_shards(attn_mask):
    """group_L[g]/group_P[g] for q-block pairs g=(2g,2g+1), maxed over batches."""
    am = np.asarray(attn_mask)
    need = np.zeros((B, NBLK), dtype=np.int64)
    for b in range(B):
        m = am[b, 0]
        anyrow = m.any(axis=1)
        rev = m[:, ::-1]
        last = np.where(anyrow, T - rev.argmax(axis=1), 1)
        for i in range(NBLK):
            need[b, i] = int(np.ceil(last[P * i:P * (i + 1)].max() / P))
    group_L, group_P = [], []
    for g in range(NG):
        L = int(max(need[b, 2 * g + j] for b in range(B) for j in range(2)))
        L = max(2, int(np.ceil(L / 2) * 2))
        group_L.append(L)
    for g in range(NG):
        p_min = group_L[g]
        for b in range(B):
            for j in range(2):
                rows = am[b, 0][P * (2 * g + j):P * (2 * g + j + 1)]
                p = 0
                while p < p_min and rows[:, P * p:P * (p + 1)].all():
                    p += 1
                p_min = min(p_min, p)
        group_P.append(p_min)
    return group_L, group_P


def make_in_maps(x, positions, attn_mask, w_qkv, w_out, group_L, group_P):
    bf = ml_dtypes.bfloat16
    x = np.asarray(x, dtype=np.float32)
    positions = np.asarray(positions)
    am = np.asarray(attn_mask)
    w_qkv = np.asarray(w_qkv, dtype=np.float32)
    w_out = np.asarray(w_out, dtype=np.float32)
    frac = 2.0 * np.arange(HD // 2, dtype=np.float32) / HD
    inv_ts = (ROPE_BASE ** frac).astype(np.float32) ** -1
    mcols = sum((L - P0) * 256 for L, P0 in zip(group_L, group_P))
    in_maps = []
    for c in range(8):
        b, kh = c // 2, c % 2
        xT = np.ascontiguousarray(x[b].T).astype(bf)
        pos_b = positions[b].astype(np.float32)
        ang = inv_ts[:, None] * pos_b[None, :]
        cos_k = np.cos(ang).astype(bf)
        sin_k = np.sin(ang).astype(bf)
        # weight slices for this head-half
        qf = slice(NQH * HD * kh, NQH * HD * (kh + 1))
        kf = slice(NH * HD + NHL * HD * kh, NH * HD + NHL * HD * (kh + 1))
        vf = slice((NH + NKV) * HD + NHL * HD * kh,
                   (NH + NKV) * HD + NHL * HD * (kh + 1))
        wq = np.ascontiguousarray(w_qkv[:, qf]).astype(bf)
        wk = np.ascontiguousarray(w_qkv[:, kf]).astype(bf)
        wv = np.ascontiguousarray(w_qkv[:, vf]).astype(bf)
        wo = np.ascontiguousarray(w_out[NQH * HD * kh:NQH * HD * (kh + 1), :]).astype(bf)
        # maskT per group: tail blocks P0..L-1, [128 w, 256 q] each
        mT = np.zeros((P, max(mcols, 256)), dtype=np.float32)
        off = 0
        for g in range(NG):
            L, P0 = group_L[g], group_P[g]
            for ti in range(L - P0):
                l = P0 + ti
                for j in range(2):
                    qb = 2 * g + j
                    sub = am[b, 0][P * qb:P * (qb + 1), P * l:P * (l + 1)]
                    mT[:, off + 256 * ti + P * j: off + 256 * ti + P * (j + 1)] = \
                        np.where(sub, 0.0, MASK_NEG).T
            off += (L - P0) * 256
        in_maps.append(dict(
            xT=xT, wq=wq, wk=wk, wv=wv, wo=wo,
            cos_k=cos_k, sin_k=sin_k, maskT=mT.astype(bf),
            c1bf=np.ones((P, 8), dtype=bf),
            c1fr=np.ones((1, P), dtype=np.float32)))
    return in_maps


def assemble(results):
    out = np.empty((B, T, D), dtype=np.float32)
    for b in range(B):
        out[b] = results[2 * b]["out"]
        out[b] += results[2 * b + 1]["out"]
    return out


# ======================= entry point =======================================

_NC_CACHE = {}


def kernel(x, positions, attn_mask, w_qkv, w_out):
    from concourse.bass_utils import run_bass_kernel_spmd
    group_L, group_P = plan_shards(attn_mask)
    key = (tuple(group_L), tuple(group_P))
    if key not in _NC_CACHE:
        _NC_CACHE[key] = build_nc(group_L, group_P)
    nc = _NC_CACHE[key]
    in_maps = make_in_maps(x, positions, attn_mask, w_qkv, w_out,
                           group_L, group_P)
    res = run_bass_kernel_spmd(nc, in_maps, list(range(8)), trace=False)
    return assemble(res.results)
